# revision 38
# baseline (speedup 1.0000x reference)
"""BarrierNet Trainium2 kernel: tiny MLP (10->128->{32,32}->{2,1}) + halfspace QP
projection over a 524288-row batch, data-parallel over 8 NeuronCores.

~129us/core (from a 158us baseline).  ACT is the pacing engine at its HW
floor: silu on 12.58M elems/core at ~0.83ns/elem/lane + ~220ns/instr over
64 instructions ~= 98us busy; PE (~90us incl ldweights) and DVE (~76us)
sit just under it.  Layout per core (65536 samples):
  - obs loaded [128p, 5120f] fp32 as one big tile (8 piece DMAs); DVE casts
    each piece to a 32-col-strided bf16 copy.  Pad columns stay
    uninitialized: the PE transposes read only the 10 real columns and L1
    reads only rows [32s, 32s+10) of the transposed strips.
  - PE transposes blocks vs a bf16 identity (plain matmul, NOT
    is_transpose: tile_position-disjoint plain matmuls overlap on real HW,
    is_transpose ones do not; XBAR dma_start_transpose is ~2.7us per
    [128,128] tile on HW, 24x its cost model -- both alternatives lose).
  - MLP runs feature-on-partition with bf16 matmuls (W stationary), SiLU on
    ACT (PSUM->SBUF, bias fused).  Emission order keeps every ACT
    instruction's inputs produced >=1 full ACT instruction ahead
    (s1(i-1) before L1(i); L2(i-2) before L1(i) on PE), so ACT never waits;
    a warmup silu hides the 1.3us ACT table load under the input DMAs.
  - PSUM (8 banks exactly): l1p 4 + l2p 2 + u3p 1 + xtp 1 (the xt bank is
    shared with the back-transpose output utp within each iteration).
  - QP epilogue on DVE in fp32 from obs cols 6..9 (strided views of the big
    obs tile), one tanh per batch on ACT (same table set as silu -> no
    table switch).  Epilogue runs in block-range batches (256/128/64/32
    wide) to amortize the ~116-cycle DVE instruction init, split into 3
    staggered parts so no DVE burst stalls the u3cast/extract chain that
    feeds the PE back-transposes.  GPSIMD is left idle on purpose: it
    cannot touch PSUM, and slow Pool ops inside cross-engine dependency
    chains stall the whole pipeline.
"""

import numpy as np
import ml_dtypes

B, F, H1, C = 524288, 10, 128, 2
NCORES = 8
BC = B // NCORES            # 65536 samples per core
P = 128
CPP = BC // P               # 512 samples per partition == blocks per core
NBLK = CPP
BLK_PER_CHUNK = 16
NCHUNK = NBLK // BLK_PER_CHUNK   # 32
R2 = 0.8 * 0.8
NPC = 8                     # obs pieces == epilogue pieces
BPP = NBLK // NPC           # 64 blocks per piece
FP = 32                     # padded feature stride

_BUILT = None


def _legalize_single_wait(nc, mybir):
    """This walrus build encodes at most ONE sync wait per instruction; split
    multi-wait instructions into preceding NoOp wait-carriers."""
    n = 0
    for f in nc.m.functions:
        for b in f.blocks:
            new_list = []
            changed = False
            for inst in b.instructions:
                si = inst.sync_info
                if si is not None and len(si.on_wait) > 1:
                    waits = list(si.on_wait)
                    for k, w in enumerate(waits[1:]):
                        new_list.append(mybir.InstNoOp(
                            name=f"{inst.name}-wsplit-{k}", engine=inst.engine,
                            ins=[], outs=[],
                            sync_info=mybir.SyncInfo(on_update=[], on_wait=[w])))
                        n += 1
                    si.on_wait = waits[:1]
                    inst.sync_info = si
                    changed = True
                new_list.append(inst)
            if changed:
                b.instructions = new_list
    return n


def _build():
    global _BUILT
    if _BUILT is not None:
        return _BUILT
    import concourse.bass as bass
    import concourse.tile as tile
    import concourse.mybir as mybir

    f32 = mybir.dt.float32
    bf16 = mybir.dt.bfloat16
    AF = mybir.ActivationFunctionType
    ALU = mybir.AluOpType

    nc = bass.Bass("TRN2")
    obs_d = nc.dram_tensor("obs", [BC, F], f32, kind="ExternalInput")
    w1t_d = nc.dram_tensor("w1t", [P, 128], bf16, kind="ExternalInput")
    w2t_d = nc.dram_tensor("w2t", [P, 64], bf16, kind="ExternalInput")
    w3_d = nc.dram_tensor("w3", [P, 4], bf16, kind="ExternalInput")
    idb_d = nc.dram_tensor("idb", [P, 128], bf16, kind="ExternalInput")
    b1_d = nc.dram_tensor("b1v", [P, 1], f32, kind="ExternalInput")
    b2_d = nc.dram_tensor("b2v", [P, 1], f32, kind="ExternalInput")
    b3_d = nc.dram_tensor("b3v", [P, 1], f32, kind="ExternalInput")
    out_d = nc.dram_tensor("out", [BC, C], f32, kind="ExternalOutput")

    obs_ap = obs_d[:].rearrange("(p c) f -> p (c f)", p=P)   # [128, 5120]
    out_ap = out_d[:].rearrange("(p c) u -> p (c u)", p=P)   # [128, 1024]

    with tile.TileContext(nc) as tc:
        from contextlib import ExitStack
        es = ExitStack()
        with es:
            cpool = es.enter_context(tc.tile_pool(name="const", bufs=1))
            bigpool = es.enter_context(tc.tile_pool(name="big", bufs=1))
            xpool = es.enter_context(tc.tile_pool(name="xts", bufs=5))
            wpool = es.enter_context(tc.tile_pool(name="work", bufs=3))
            epool = es.enter_context(tc.tile_pool(name="epi", bufs=1))
            ppool = es.enter_context(tc.tile_pool(name="ps", bufs=1, space="PSUM"))

            # ---- obs as single big tiles; pieces are column ranges ----
            obsf_all = bigpool.tile([P, NBLK * F], f32, tag="obsf",
                                    name="obsf")       # [128, 5120]
            obsb_all = bigpool.tile([P, NBLK * FP], bf16, tag="obsb",
                                    name="obsb")       # [128, 16384]
            obv_all = obsb_all[:].rearrange("p (c f) -> p c f", f=FP)
            # NOTE: pad columns [F:FP] are never initialized -- the PE
            # transposes read only cols [0:F) of each block, and L1 reads
            # only rows [32s, 32s+10) of the transposed strips.

            def load_piece(t):
                nc.sync.dma_start(
                    out=obsf_all[:, BPP * F * t:BPP * F * (t + 1)],
                    in_=obs_ap[:, BPP * F * t:BPP * F * (t + 1)])

            load_piece(0)

            # ---- constants (SP ring, right behind the first obs piece) ----
            idb = cpool.tile([P, 128], bf16)
            nc.sync.dma_start(out=idb, in_=idb_d[:])
            # warmup: trigger the SILU ACT table load while DMAs stream in,
            # so the ~1.3us load isn't on the first real silu's critical path
            actwarm = cpool.tile([P, 1], f32)
            nc.scalar.activation(out=actwarm, in_=idb[:, 0:1], func=AF.Silu)
            w1t = cpool.tile([P, 128], bf16)
            nc.sync.dma_start(out=w1t, in_=w1t_d[:])
            b1s = cpool.tile([P, 1], f32)
            nc.sync.dma_start(out=b1s, in_=b1_d[:])
            w2t = cpool.tile([P, 64], bf16)
            nc.sync.dma_start(out=w2t, in_=w2t_d[:])
            b2s = cpool.tile([P, 1], f32)
            nc.sync.dma_start(out=b2s, in_=b2_d[:])
            w3s = cpool.tile([P, 4], bf16)
            nc.sync.dma_start(out=w3s, in_=w3_d[:])
            b3s = cpool.tile([P, 1], f32)
            nc.sync.dma_start(out=b3s, in_=b3_d[:])

            for t in range(1, NPC):
                load_piece(t)

            ubuf = bigpool.tile([P, CPP * 3], f32)        # (u0,u1,apre)
            outt = bigpool.tile([P, CPP * C], f32)
            u0v = ubuf[:, 0::3]
            u1v = ubuf[:, 1::3]
            apv = ubuf[:, 2::3]

            def et(tag):
                return epool.tile([P, CPP], f32, tag=tag, name=tag)

            ss_t, t0_t, ggc_t, rec_t, b1p_t, e1_t, t2_t = (
                et("ss"), et("t0"), et("ggc"), et("rec"), et("b1p"), et("e1"),
                et("t2"))
            th_t, d0_t, c0_t, d1_t, c1_t, cc_t, ff_t, mn_t, q2_t, s0_t, s1_t = (
                et("th"), et("d0"), et("c0"), et("d1"), et("c1"), et("cc"),
                et("ff"), et("mn"), et("q2"), et("s0"), et("s1"))

            def epiB1(c0, c1):
                """Epilogue part 1 for blocks [c0, c1): obs-only terms on
                GPSIMD + tanh on ACT.  The tile scheduler hoists
                dependency-free ops to the front of each engine's stream;
                on DVE that congested the pipeline-fill phase, so the
                obs-only prefix lives on the otherwise-idle GPSIMD where
                early hoisting is free."""
                sl = slice(c0, c1)
                rx = obsf_all[:, 6::F][:, sl]
                ry = obsf_all[:, 7::F][:, sl]
                ss, t0, ggc, b1p = (
                    x[:, sl] for x in (ss_t, t0_t, ggc_t, b1p_t))
                th, ap_ = th_t[:, sl], apv[:, sl]
                GT = nc.gpsimd.tensor_tensor
                GT(out=ss, in0=rx, in1=rx, op=ALU.mult)
                GT(out=t0, in0=ry, in1=ry, op=ALU.mult)
                GT(out=ss, in0=ss, in1=t0, op=ALU.add)
                nc.gpsimd.tensor_scalar(out=ggc, in0=ss, scalar1=4.0,
                                        scalar2=1e-12, op0=ALU.mult,
                                        op1=ALU.max)
                nc.gpsimd.tensor_single_scalar(out=b1p, in_=ss, scalar=R2,
                                               op=ALU.subtract)
                nc.vector.reciprocal(out=rec_t[:, sl], in_=ggc)
                nc.scalar.activation(out=th, in_=ap_, func=AF.Tanh, scale=0.5)

            def epiB2a(c0, c1):
                """Epilogue part 2a for blocks [c0, c1)."""
                sl = slice(c0, c1)
                rx = obsf_all[:, 6::F][:, sl]
                ry = obsf_all[:, 7::F][:, sl]
                vx = obsf_all[:, 8::F][:, sl]
                vy = obsf_all[:, 9::F][:, sl]
                b1p, t2 = b1p_t[:, sl], t2_t[:, sl]
                th, d0, c0_, d1, c1_, cc, ff, mn = (
                    x[:, sl] for x in (th_t, d0_t, c0_t, d1_t, c1_t, cc_t,
                                       ff_t, mn_t))
                u0, u1 = u0v[:, sl], u1v[:, sl]
                TT = nc.vector.tensor_tensor
                # t2 = b1p * (1 + th)  == b1p + th*b1p
                nc.vector.scalar_tensor_tensor(out=t2, in0=th, scalar=1.0,
                                               in1=b1p, op0=ALU.add,
                                               op1=ALU.mult)
                TT(out=d0, in0=u0, in1=vx, op=ALU.subtract)
                TT(out=c0_, in0=rx, in1=d0, op=ALU.mult)
                TT(out=d1, in0=u1, in1=vy, op=ALU.subtract)
                TT(out=c1_, in0=ry, in1=d1, op=ALU.mult)
                TT(out=cc, in0=c0_, in1=c1_, op=ALU.add)
                TT(out=ff, in0=cc, in1=t2, op=ALU.add)
                nc.vector.tensor_single_scalar(out=mn, in_=ff, scalar=0.0,
                                               op=ALU.min)

            def epiB2b(c0, c1):
                """Epilogue part 2b + store for blocks [c0, c1)."""
                sl = slice(c0, c1)
                rx = obsf_all[:, 6::F][:, sl]
                ry = obsf_all[:, 7::F][:, sl]
                rec, mn, q2, s0, s1 = (
                    x[:, sl] for x in (rec_t, mn_t, q2_t, s0_t, s1_t))
                u0, u1 = u0v[:, sl], u1v[:, sl]
                TT = nc.vector.tensor_tensor
                TT(out=q2, in0=mn, in1=rec, op=ALU.mult)
                TT(out=s0, in0=q2, in1=rx, op=ALU.mult)
                nc.vector.scalar_tensor_tensor(out=outt[:, 0::2][:, sl], in0=s0,
                                               scalar=-4.0, in1=u0,
                                               op0=ALU.mult, op1=ALU.add)
                TT(out=s1, in0=q2, in1=ry, op=ALU.mult)
                nc.vector.scalar_tensor_tensor(out=outt[:, 1::2][:, sl], in0=s1,
                                               scalar=-4.0, in1=u1,
                                               op0=ALU.mult, op1=ALU.add)
                nc.sync.dma_start(out=out_ap[:, 2 * c0:2 * c1],
                                  in_=outt[:, 2 * c0:2 * c1])

            cast_done = set()

            def cast_piece(t):
                """fp32 piece -> padded bf16 copy (DVE), then epiA on GPSIMD."""
                if t in cast_done:
                    return
                cast_done.add(t)
                nc.vector.tensor_copy(
                    out=obv_all[:, BPP * t:BPP * (t + 1), 0:F],
                    in_=obsf_all[:, BPP * F * t:BPP * F * (t + 1)].rearrange(
                        "p (c f) -> p c f", f=F))

            # ---- stages ----
            def stageA1(q):
                """PE transpose of chunk q's obs blocks -> xts bf16 SBUF."""
                qloc = q % 4
                base = (q // 4) * BPP * FP
                xtp = ppool.tile([P, 512], f32, tag="xt", name="xtp")
                for jj in range(BLK_PER_CHUNK):
                    s, m = jj % 4, jj // 4
                    jloc = qloc * 16 + 4 * m + s
                    nc.tensor.matmul(
                        out=xtp[32 * s:32 * s + 10, 128 * m:128 * m + 128],
                        lhsT=obsb_all[:, base + jloc * FP:base + jloc * FP + F],
                        rhs=idb[:],
                        tile_position=(0, 32 * s),
                    )
                xts = xpool.tile([P, 512], bf16, tag="xts", name="xts")
                nc.vector.tensor_copy(out=xts[:], in_=xtp[:])
                return xts

            def stageL1(q, xts):
                l1p = ppool.tile([P, 2048], f32, tag="l1", name="l1p")
                for s in range(4):
                    nc.tensor.matmul(
                        out=l1p[:, 512 * s:512 * (s + 1)],
                        lhsT=w1t[32 * s:32 * s + 10, :],
                        rhs=xts[32 * s:32 * s + 10, :],
                        tile_position=(32 * s, 0),
                    )
                return l1p

            def stageS1(q, l1p):
                h1 = wpool.tile([P, 2048], bf16, tag="h1", name="h1")
                nc.scalar.activation(out=h1[:], in_=l1p[:], func=AF.Silu,
                                     bias=b1s[:, 0:1], scale=1.0)
                return h1

            def stageL2(q, h1):
                l2p = ppool.tile([P, 1024], f32, tag="l2", name="l2p")
                for g in range(4):
                    pb = 64 * (g % 2)
                    nc.tensor.matmul(
                        out=l2p[pb:pb + 64, 512 * (g // 2):512 * (g // 2) + 512],
                        lhsT=w2t[:],
                        rhs=h1[:, 512 * g:512 * (g + 1)],
                        tile_position=(0, pb),
                    )
                return l2p

            def stageS2(q, l2p):
                x2 = wpool.tile([P, 1024], bf16, tag="x2", name="x2")
                nc.scalar.activation(out=x2[:], in_=l2p[:], func=AF.Silu,
                                     bias=b2s[:, 0:1], scale=1.0)
                return x2

            def stageL3(q, x2):
                u3p = ppool.tile([P, 512], f32, tag="u3", name="u3p")
                for g in range(4):
                    pb = 64 * (g % 2)
                    nc.tensor.matmul(
                        out=u3p[32 * g:32 * g + 3, :],
                        lhsT=w3s[pb:pb + 64, 0:3],
                        rhs=x2[pb:pb + 64, 512 * (g // 2):512 * (g // 2) + 512],
                        tile_position=(pb, 32 * g),
                    )
                u3s = wpool.tile([P, 512], bf16, tag="u3s", name="u3s")
                nc.vector.tensor_scalar_add(out=u3s[:], in0=u3p[:],
                                            scalar1=b3s[:, 0:1])
                return u3s

            def stageOut(q, u3s):
                """PE back-transpose (into the xt PSUM bank) -> extract."""
                utp = ppool.tile([P, 512], bf16, tag="xt", name="utp")
                for b in range(4):
                    nc.tensor.transpose(
                        out=utp[:, 128 * b:128 * (b + 1)],
                        in_=u3s[:, 128 * b:128 * (b + 1)],
                        identity=idb[:],
                    )
                src = utp[:].rearrange("p (b g x) -> p b g x", b=4, g=4)[:, :, :, 0:3]
                dst = ubuf[:, 48 * q:48 * (q + 1)].rearrange(
                    "p (b g f) -> p b g f", b=4, g=4)
                nc.vector.tensor_copy(out=dst, in_=src)
                # epilogue batches (block ranges of 512): big early batches
                # amortize DVE op init; small at the tail to shorten the
                # drain; parts staggered across iterations so DVE bursts
                # stay short.  A batch's epiB1 at trigger q needs extracts
                # of chunks < 16*(q+1) >= c1.
                for trig, fn, c0, c1 in (
                        (15, epiB1, 0, 256), (16, epiB2a, 0, 256),
                        (17, epiB2b, 0, 256),
                        (23, epiB1, 256, 384), (24, epiB2a, 256, 384),
                        (25, epiB2b, 256, 384),
                        (27, epiB1, 384, 448), (28, epiB2a, 384, 448),
                        (29, epiB2b, 384, 448), (29, epiB1, 448, 480),
                        (30, epiB2a, 448, 480), (30, epiB2b, 448, 480),
                        (31, epiB1, 480, 512), (31, epiB2a, 480, 512),
                        (31, epiB2b, 480, 512)):
                    if q == trig:
                        fn(c0, c1)

            # ---- main loop ----
            # iteration i:
            #   ACT: s1(i-1), s2(i-2)          [+ tanh inside epi(i-4 bnd)]
            #   PE:  L2(i-2), A1(i+1), L1(i), L3(i-3), Tout(i-4)
            #   DVE: xts-cast(i+1), u3 cast(i-3), extract(i-4) [+ epilogue]
            # ACT instructions always have inputs produced >=1 full ACT
            # instruction earlier, so ACT paces the kernel without waiting;
            # PE fills silu time with transposes.
            # cast piece 0 up front; pieces 1..7 one per early iteration so
            # the DVE queue stays smooth (piece t needed from chunk 4t)
            cast_piece(0)

            # prologue: prime 3 transpose chunks so the early cadence has
            # xts ready the moment each silu frees l1p
            xts_d, l1p_d, h1_d, l2p_d, x2_d, u3s_d = {}, {}, {}, {}, {}, {}
            xts_d[0] = stageA1(0)
            for i in range(NCHUNK + 4):
                if 1 + i // 2 < NPC and i % 2 == 0:
                    cast_piece(1 + i // 2)
                if 1 <= i <= NCHUNK:
                    h1_d[i - 1] = stageS1(i - 1, l1p_d.pop(i - 1))
                if 2 <= i <= NCHUNK + 1:
                    l2p_d[i - 2] = stageL2(i - 2, h1_d.pop(i - 2))
                if i + 1 < NCHUNK:
                    xts_d[i + 1] = stageA1(i + 1)
                if i < NCHUNK:
                    l1p_d[i] = stageL1(i, xts_d.pop(i))
                if 2 <= i <= NCHUNK + 1:
                    x2_d[i - 2] = stageS2(i - 2, l2p_d.pop(i - 2))
                if 3 <= i <= NCHUNK + 2:
                    u3s_d[i - 3] = stageL3(i - 3, x2_d.pop(i - 3))
                if 4 <= i <= NCHUNK + 3:
                    stageOut(i - 4, u3s_d.pop(i - 4))

    _legalize_single_wait(nc, mybir)
    _BUILT = nc
    return nc


def _const_inputs(inputs):
    bf = ml_dtypes.bfloat16
    W1 = np.asarray(inputs["W1"], np.float32)     # [128, 10]
    b1 = np.asarray(inputs["b1"], np.float32)     # [128]
    W21 = np.asarray(inputs["W21"], np.float32)   # [32, 128]
    b21 = np.asarray(inputs["b21"], np.float32)
    W22 = np.asarray(inputs["W22"], np.float32)
    b22 = np.asarray(inputs["b22"], np.float32)
    W31 = np.asarray(inputs["W31"], np.float32)   # [2, 32]
    b31 = np.asarray(inputs["b31"], np.float32)
    W32 = np.asarray(inputs["W32"], np.float32)   # [1, 32]
    b32 = np.asarray(inputs["b32"], np.float32)

    w1t = np.zeros((P, 128), np.float32)
    for s in range(4):
        w1t[32 * s:32 * s + 10, :] = W1.T
    w2t = np.zeros((P, 64), np.float32)
    w2t[:, 0:32] = W21.T
    w2t[:, 32:64] = W22.T
    w3 = np.zeros((P, 4), np.float32)
    w3[0:32, 0:2] = W31.T
    w3[32:64, 2] = W32[0, :]
    w3[64:96, 0:2] = W31.T
    w3[96:128, 2] = W32[0, :]
    b1v = b1.reshape(P, 1)
    b2v = np.concatenate([b21, b22, b21, b22]).reshape(P, 1)
    b3 = np.array([b31[0], b31[1], b32[0]], np.float32)
    b3v = np.zeros((P, 1), np.float32)
    for g in range(4):
        b3v[32 * g:32 * g + 3, 0] = b3
    idb = np.eye(128, dtype=np.float32)
    return {
        "w1t": w1t.astype(bf), "w2t": w2t.astype(bf), "w3": w3.astype(bf),
        "idb": idb.astype(bf),
        "b1v": b1v, "b2v": b2v, "b3v": b3v,
    }


def kernel(**inputs):
    import time
    from concourse.bass_utils import run_bass_kernel_spmd
    obs = np.ascontiguousarray(np.asarray(inputs["obs"], np.float32))
    nc = _build()
    consts = _const_inputs(inputs)
    in_maps = []
    for c in range(NCORES):
        m = {"obs": obs[c * BC:(c + 1) * BC]}
        m.update(consts)
        in_maps.append(m)
    last_err = None
    for attempt in range(3):
        try:
            res = run_bass_kernel_spmd(nc, in_maps, core_ids=list(range(NCORES)))
            break
        except Exception as e:  # transient device/tunnel flakiness: retry
            last_err = e
            time.sleep(3.0)
    else:
        raise last_err
    out = np.concatenate([res.results[c]["out"] for c in range(NCORES)], axis=0)
    return out


# revision 39
# speedup vs baseline: 1.1181x; 1.1181x over previous
"""BarrierNet Trainium2 kernel: tiny MLP (10->128->{32,32}->{2,1}) + halfspace QP
projection over a 524288-row batch, data-parallel over 8 NeuronCores.

~129us/core (from a 158us baseline).  ACT is the pacing engine at its HW
floor: silu on 12.58M elems/core at ~0.83ns/elem/lane + ~220ns/instr over
64 instructions ~= 98us busy; PE (~90us incl ldweights) and DVE (~76us)
sit just under it.  Layout per core (65536 samples):
  - obs loaded [128p, 5120f] fp32 as one big tile (8 piece DMAs); DVE casts
    each piece to a 32-col-strided bf16 copy.  Pad columns stay
    uninitialized: the PE transposes read only the 10 real columns and L1
    reads only rows [32s, 32s+10) of the transposed strips.
  - PE transposes blocks vs a bf16 identity (plain matmul, NOT
    is_transpose: tile_position-disjoint plain matmuls overlap on real HW,
    is_transpose ones do not; XBAR dma_start_transpose is ~2.7us per
    [128,128] tile on HW, 24x its cost model -- both alternatives lose).
  - MLP runs feature-on-partition with bf16 matmuls (W stationary), SiLU on
    ACT (PSUM->SBUF, bias fused).  Emission order keeps every ACT
    instruction's inputs produced >=1 full ACT instruction ahead
    (s1(i-1) before L1(i); L2(i-2) before L1(i) on PE), so ACT never waits;
    a warmup silu hides the 1.3us ACT table load under the input DMAs.
  - PSUM (8 banks exactly): l1p 4 + l2p 2 + u3p 1 + xtp 1 (the xt bank is
    shared with the back-transpose output utp within each iteration).
  - QP epilogue on DVE in fp32 from obs cols 6..9 (strided views of the big
    obs tile), one tanh per batch on ACT (same table set as silu -> no
    table switch).  Epilogue runs in block-range batches (256/128/64/32
    wide) to amortize the ~116-cycle DVE instruction init, split into 3
    staggered parts so no DVE burst stalls the u3cast/extract chain that
    feeds the PE back-transposes.  GPSIMD is left idle on purpose: it
    cannot touch PSUM, and slow Pool ops inside cross-engine dependency
    chains stall the whole pipeline.
"""

import numpy as np
import ml_dtypes

B, F, H1, C = 524288, 10, 128, 2
NCORES = 8
BC = B // NCORES            # 65536 samples per core
P = 128
CPP = BC // P               # 512 samples per partition == blocks per core
NBLK = CPP
BLK_PER_CHUNK = 16
NCHUNK = NBLK // BLK_PER_CHUNK   # 32
R2 = 0.8 * 0.8
NPC = 8                     # obs pieces == epilogue pieces
BPP = NBLK // NPC           # 64 blocks per piece
FP = 32                     # padded feature stride

_BUILT = None


def _legalize_single_wait(nc, mybir):
    """This walrus build encodes at most ONE sync wait per instruction; split
    multi-wait instructions into preceding NoOp wait-carriers."""
    n = 0
    for f in nc.m.functions:
        for b in f.blocks:
            new_list = []
            changed = False
            for inst in b.instructions:
                si = inst.sync_info
                if si is not None and len(si.on_wait) > 1:
                    waits = list(si.on_wait)
                    for k, w in enumerate(waits[1:]):
                        new_list.append(mybir.InstNoOp(
                            name=f"{inst.name}-wsplit-{k}", engine=inst.engine,
                            ins=[], outs=[],
                            sync_info=mybir.SyncInfo(on_update=[], on_wait=[w])))
                        n += 1
                    si.on_wait = waits[:1]
                    inst.sync_info = si
                    changed = True
                new_list.append(inst)
            if changed:
                b.instructions = new_list
    return n


def _build():
    global _BUILT
    if _BUILT is not None:
        return _BUILT
    import concourse.bass as bass
    import concourse.tile as tile
    import concourse.mybir as mybir

    f32 = mybir.dt.float32
    bf16 = mybir.dt.bfloat16
    AF = mybir.ActivationFunctionType
    ALU = mybir.AluOpType

    nc = bass.Bass("TRN2")
    obs_d = nc.dram_tensor("obs", [BC, F], f32, kind="ExternalInput")
    w1t_d = nc.dram_tensor("w1t", [P, 128], bf16, kind="ExternalInput")
    w2t_d = nc.dram_tensor("w2t", [P, 64], bf16, kind="ExternalInput")
    w3_d = nc.dram_tensor("w3", [P, 4], bf16, kind="ExternalInput")
    idb_d = nc.dram_tensor("idb", [P, 128], bf16, kind="ExternalInput")
    b1_d = nc.dram_tensor("b1v", [P, 1], f32, kind="ExternalInput")
    b2_d = nc.dram_tensor("b2v", [P, 1], f32, kind="ExternalInput")
    b3_d = nc.dram_tensor("b3v", [P, 1], f32, kind="ExternalInput")
    out_d = nc.dram_tensor("out", [BC, C], f32, kind="ExternalOutput")

    obs_ap = obs_d[:].rearrange("(p c) f -> p (c f)", p=P)   # [128, 5120]
    out_ap = out_d[:].rearrange("(p c) u -> p (c u)", p=P)   # [128, 1024]

    with tile.TileContext(nc) as tc:
        from contextlib import ExitStack
        es = ExitStack()
        with es:
            cpool = es.enter_context(tc.tile_pool(name="const", bufs=1))
            bigpool = es.enter_context(tc.tile_pool(name="big", bufs=1))
            xpool = es.enter_context(tc.tile_pool(name="xts", bufs=5))
            wpool = es.enter_context(tc.tile_pool(name="work", bufs=3))
            epool = es.enter_context(tc.tile_pool(name="epi", bufs=1))
            ppool = es.enter_context(tc.tile_pool(name="ps", bufs=1, space="PSUM"))

            # ---- obs as single big tiles; pieces are column ranges ----
            obsf_all = bigpool.tile([P, NBLK * F], f32, tag="obsf",
                                    name="obsf")       # [128, 5120]
            obsb_all = bigpool.tile([P, NBLK * FP], bf16, tag="obsb",
                                    name="obsb")       # [128, 16384]
            obv_all = obsb_all[:].rearrange("p (c f) -> p c f", f=FP)
            # NOTE: pad columns [F:FP] are never initialized -- the PE
            # transposes read only cols [0:F) of each block, and L1 reads
            # only rows [32s, 32s+10) of the transposed strips.

            def load_piece(t):
                nc.sync.dma_start(
                    out=obsf_all[:, BPP * F * t:BPP * F * (t + 1)],
                    in_=obs_ap[:, BPP * F * t:BPP * F * (t + 1)])

            load_piece(0)

            # ---- constants (SP ring, right behind the first obs piece) ----
            idb = cpool.tile([P, 128], bf16)
            nc.sync.dma_start(out=idb, in_=idb_d[:])
            # warmup: trigger the SILU ACT table load while DMAs stream in,
            # so the ~1.3us load isn't on the first real silu's critical path
            actwarm = cpool.tile([P, 1], f32)
            nc.scalar.activation(out=actwarm, in_=idb[:, 0:1], func=AF.Silu)
            w1t = cpool.tile([P, 128], bf16)
            nc.sync.dma_start(out=w1t, in_=w1t_d[:])
            b1s = cpool.tile([P, 1], f32)
            nc.sync.dma_start(out=b1s, in_=b1_d[:])
            w2t = cpool.tile([P, 64], bf16)
            nc.sync.dma_start(out=w2t, in_=w2t_d[:])
            b2s = cpool.tile([P, 1], f32)
            nc.sync.dma_start(out=b2s, in_=b2_d[:])
            w3s = cpool.tile([P, 4], bf16)
            nc.sync.dma_start(out=w3s, in_=w3_d[:])
            b3s = cpool.tile([P, 1], f32)
            nc.sync.dma_start(out=b3s, in_=b3_d[:])

            for t in range(1, NPC):
                load_piece(t)

            ubuf = bigpool.tile([P, CPP * 3], f32)        # (u0,u1,apre)
            outt = bigpool.tile([P, CPP * C], f32)
            u0v = ubuf[:, 0::3]
            u1v = ubuf[:, 1::3]
            apv = ubuf[:, 2::3]

            def et(tag):
                return epool.tile([P, CPP], f32, tag=tag, name=tag)

            ss_t, t0_t, ggc_t, rec_t, b1p_t, e1_t, t2_t = (
                et("ss"), et("t0"), et("ggc"), et("rec"), et("b1p"), et("e1"),
                et("t2"))
            th_t, d0_t, c0_t, d1_t, c1_t, cc_t, ff_t, mn_t, q2_t, s0_t, s1_t = (
                et("th"), et("d0"), et("c0"), et("d1"), et("c1"), et("cc"),
                et("ff"), et("mn"), et("q2"), et("s0"), et("s1"))

            def epiB1(c0, c1):
                """Epilogue part 1 for blocks [c0, c1): obs-only terms +
                tanh.  All on DVE: the tile scheduler hoists these
                dependency-free ops into the pipeline-fill phase (~1.8us of
                early DVE), but moving them to GPSIMD measurably stalls the
                whole pipeline (slow Pool ops in the dependency fabric), so
                DVE it is."""
                sl = slice(c0, c1)
                rx = obsf_all[:, 6::F][:, sl]
                ry = obsf_all[:, 7::F][:, sl]
                ss, t0, ggc, rec, b1p = (
                    x[:, sl] for x in (ss_t, t0_t, ggc_t, rec_t, b1p_t))
                th, ap_ = th_t[:, sl], apv[:, sl]
                TT = nc.vector.tensor_tensor
                TT(out=ss, in0=rx, in1=rx, op=ALU.mult)
                TT(out=t0, in0=ry, in1=ry, op=ALU.mult)
                TT(out=ss, in0=ss, in1=t0, op=ALU.add)
                nc.vector.tensor_scalar(out=ggc, in0=ss, scalar1=4.0,
                                        scalar2=1e-12, op0=ALU.mult,
                                        op1=ALU.max)
                nc.vector.reciprocal(out=rec, in_=ggc)
                nc.vector.tensor_single_scalar(out=b1p, in_=ss, scalar=R2,
                                               op=ALU.subtract)
                nc.scalar.activation(out=th, in_=ap_, func=AF.Tanh, scale=0.5)

            def epiB2a(c0, c1):
                """Epilogue part 2a for blocks [c0, c1)."""
                sl = slice(c0, c1)
                rx = obsf_all[:, 6::F][:, sl]
                ry = obsf_all[:, 7::F][:, sl]
                vx = obsf_all[:, 8::F][:, sl]
                vy = obsf_all[:, 9::F][:, sl]
                b1p, t2 = b1p_t[:, sl], t2_t[:, sl]
                th, d0, c0_, d1, c1_, cc, ff, mn = (
                    x[:, sl] for x in (th_t, d0_t, c0_t, d1_t, c1_t, cc_t,
                                       ff_t, mn_t))
                u0, u1 = u0v[:, sl], u1v[:, sl]
                TT = nc.vector.tensor_tensor
                # t2 = b1p * (1 + th)  == b1p + th*b1p
                nc.vector.scalar_tensor_tensor(out=t2, in0=th, scalar=1.0,
                                               in1=b1p, op0=ALU.add,
                                               op1=ALU.mult)
                TT(out=d0, in0=u0, in1=vx, op=ALU.subtract)
                TT(out=c0_, in0=rx, in1=d0, op=ALU.mult)
                TT(out=d1, in0=u1, in1=vy, op=ALU.subtract)
                TT(out=c1_, in0=ry, in1=d1, op=ALU.mult)
                TT(out=cc, in0=c0_, in1=c1_, op=ALU.add)
                TT(out=ff, in0=cc, in1=t2, op=ALU.add)
                nc.vector.tensor_single_scalar(out=mn, in_=ff, scalar=0.0,
                                               op=ALU.min)

            def epiB2b(c0, c1):
                """Epilogue part 2b + store for blocks [c0, c1)."""
                sl = slice(c0, c1)
                rx = obsf_all[:, 6::F][:, sl]
                ry = obsf_all[:, 7::F][:, sl]
                rec, mn, q2, s0, s1 = (
                    x[:, sl] for x in (rec_t, mn_t, q2_t, s0_t, s1_t))
                u0, u1 = u0v[:, sl], u1v[:, sl]
                TT = nc.vector.tensor_tensor
                TT(out=q2, in0=mn, in1=rec, op=ALU.mult)
                TT(out=s0, in0=q2, in1=rx, op=ALU.mult)
                nc.vector.scalar_tensor_tensor(out=outt[:, 0::2][:, sl], in0=s0,
                                               scalar=-4.0, in1=u0,
                                               op0=ALU.mult, op1=ALU.add)
                TT(out=s1, in0=q2, in1=ry, op=ALU.mult)
                nc.vector.scalar_tensor_tensor(out=outt[:, 1::2][:, sl], in0=s1,
                                               scalar=-4.0, in1=u1,
                                               op0=ALU.mult, op1=ALU.add)
                nc.sync.dma_start(out=out_ap[:, 2 * c0:2 * c1],
                                  in_=outt[:, 2 * c0:2 * c1])

            cast_done = set()

            def cast_piece(t):
                """fp32 piece -> padded bf16 copy (DVE), then epiA on GPSIMD."""
                if t in cast_done:
                    return
                cast_done.add(t)
                nc.vector.tensor_copy(
                    out=obv_all[:, BPP * t:BPP * (t + 1), 0:F],
                    in_=obsf_all[:, BPP * F * t:BPP * F * (t + 1)].rearrange(
                        "p (c f) -> p c f", f=F))

            # ---- stages ----
            def stageA1(q):
                """PE transpose of chunk q's obs blocks -> xts bf16 SBUF."""
                qloc = q % 4
                base = (q // 4) * BPP * FP
                xtp = ppool.tile([P, 512], f32, tag="xt", name="xtp")
                for jj in range(BLK_PER_CHUNK):
                    s, m = jj % 4, jj // 4
                    jloc = qloc * 16 + 4 * m + s
                    nc.tensor.matmul(
                        out=xtp[32 * s:32 * s + 10, 128 * m:128 * m + 128],
                        lhsT=obsb_all[:, base + jloc * FP:base + jloc * FP + F],
                        rhs=idb[:],
                        tile_position=(0, 32 * s),
                    )
                xts = xpool.tile([P, 512], bf16, tag="xts", name="xts")
                nc.vector.tensor_copy(out=xts[:], in_=xtp[:])
                return xts

            def stageL1(q, xts):
                l1p = ppool.tile([P, 2048], f32, tag="l1", name="l1p")
                for s in range(4):
                    nc.tensor.matmul(
                        out=l1p[:, 512 * s:512 * (s + 1)],
                        lhsT=w1t[32 * s:32 * s + 10, :],
                        rhs=xts[32 * s:32 * s + 10, :],
                        tile_position=(32 * s, 0),
                    )
                return l1p

            def stageS1(q, l1p):
                h1 = wpool.tile([P, 2048], bf16, tag="h1", name="h1")
                nc.scalar.activation(out=h1[:], in_=l1p[:], func=AF.Silu,
                                     bias=b1s[:, 0:1], scale=1.0)
                return h1

            def stageL2(q, h1):
                l2p = ppool.tile([P, 1024], f32, tag="l2", name="l2p")
                for g in range(4):
                    pb = 64 * (g % 2)
                    nc.tensor.matmul(
                        out=l2p[pb:pb + 64, 512 * (g // 2):512 * (g // 2) + 512],
                        lhsT=w2t[:],
                        rhs=h1[:, 512 * g:512 * (g + 1)],
                        tile_position=(0, pb),
                    )
                return l2p

            def stageS2(q, l2p):
                x2 = wpool.tile([P, 1024], bf16, tag="x2", name="x2")
                nc.scalar.activation(out=x2[:], in_=l2p[:], func=AF.Silu,
                                     bias=b2s[:, 0:1], scale=1.0)
                return x2

            def stageL3(q, x2):
                u3p = ppool.tile([P, 512], f32, tag="u3", name="u3p")
                for g in range(4):
                    pb = 64 * (g % 2)
                    nc.tensor.matmul(
                        out=u3p[32 * g:32 * g + 3, :],
                        lhsT=w3s[pb:pb + 64, 0:3],
                        rhs=x2[pb:pb + 64, 512 * (g // 2):512 * (g // 2) + 512],
                        tile_position=(pb, 32 * g),
                    )
                u3s = wpool.tile([P, 512], bf16, tag="u3s", name="u3s")
                nc.vector.tensor_scalar_add(out=u3s[:], in0=u3p[:],
                                            scalar1=b3s[:, 0:1])
                return u3s

            def stageOut(q, u3s):
                """PE back-transpose (into the xt PSUM bank) -> extract."""
                utp = ppool.tile([P, 512], bf16, tag="xt", name="utp")
                for b in range(4):
                    nc.tensor.transpose(
                        out=utp[:, 128 * b:128 * (b + 1)],
                        in_=u3s[:, 128 * b:128 * (b + 1)],
                        identity=idb[:],
                    )
                src = utp[:].rearrange("p (b g x) -> p b g x", b=4, g=4)[:, :, :, 0:3]
                dst = ubuf[:, 48 * q:48 * (q + 1)].rearrange(
                    "p (b g f) -> p b g f", b=4, g=4)
                nc.vector.tensor_copy(out=dst, in_=src)
                # epilogue batches (block ranges of 512): big early batches
                # amortize DVE op init; small at the tail to shorten the
                # drain; parts staggered across iterations so DVE bursts
                # stay short.  A batch's epiB1 at trigger q needs extracts
                # of chunks < 16*(q+1) >= c1.
                for trig, fn, c0, c1 in (
                        (15, epiB1, 0, 256), (16, epiB2a, 0, 256),
                        (17, epiB2b, 0, 256),
                        (23, epiB1, 256, 384), (24, epiB2a, 256, 384),
                        (25, epiB2b, 256, 384),
                        (27, epiB1, 384, 448), (28, epiB2a, 384, 448),
                        (29, epiB2b, 384, 448), (29, epiB1, 448, 480),
                        (30, epiB2a, 448, 480), (30, epiB2b, 448, 480),
                        (31, epiB1, 480, 512), (31, epiB2a, 480, 512),
                        (31, epiB2b, 480, 512)):
                    if q == trig:
                        fn(c0, c1)

            # ---- main loop ----
            # iteration i:
            #   ACT: s1(i-1), s2(i-2)          [+ tanh inside epi(i-4 bnd)]
            #   PE:  L2(i-2), A1(i+1), L1(i), L3(i-3), Tout(i-4)
            #   DVE: xts-cast(i+1), u3 cast(i-3), extract(i-4) [+ epilogue]
            # ACT instructions always have inputs produced >=1 full ACT
            # instruction earlier, so ACT paces the kernel without waiting;
            # PE fills silu time with transposes.
            # cast piece 0 up front; pieces 1..7 one per early iteration so
            # the DVE queue stays smooth (piece t needed from chunk 4t)
            cast_piece(0)

            # prologue: prime 3 transpose chunks so the early cadence has
            # xts ready the moment each silu frees l1p
            xts_d, l1p_d, h1_d, l2p_d, x2_d, u3s_d = {}, {}, {}, {}, {}, {}
            xts_d[0] = stageA1(0)
            for i in range(NCHUNK + 4):
                if 1 + i // 2 < NPC and i % 2 == 0:
                    cast_piece(1 + i // 2)
                if 1 <= i <= NCHUNK:
                    h1_d[i - 1] = stageS1(i - 1, l1p_d.pop(i - 1))
                if 2 <= i <= NCHUNK + 1:
                    l2p_d[i - 2] = stageL2(i - 2, h1_d.pop(i - 2))
                if i + 1 < NCHUNK:
                    xts_d[i + 1] = stageA1(i + 1)
                if i < NCHUNK:
                    l1p_d[i] = stageL1(i, xts_d.pop(i))
                if 2 <= i <= NCHUNK + 1:
                    x2_d[i - 2] = stageS2(i - 2, l2p_d.pop(i - 2))
                if 3 <= i <= NCHUNK + 2:
                    u3s_d[i - 3] = stageL3(i - 3, x2_d.pop(i - 3))
                if 4 <= i <= NCHUNK + 3:
                    stageOut(i - 4, u3s_d.pop(i - 4))

    _legalize_single_wait(nc, mybir)
    _BUILT = nc
    return nc


def _const_inputs(inputs):
    bf = ml_dtypes.bfloat16
    W1 = np.asarray(inputs["W1"], np.float32)     # [128, 10]
    b1 = np.asarray(inputs["b1"], np.float32)     # [128]
    W21 = np.asarray(inputs["W21"], np.float32)   # [32, 128]
    b21 = np.asarray(inputs["b21"], np.float32)
    W22 = np.asarray(inputs["W22"], np.float32)
    b22 = np.asarray(inputs["b22"], np.float32)
    W31 = np.asarray(inputs["W31"], np.float32)   # [2, 32]
    b31 = np.asarray(inputs["b31"], np.float32)
    W32 = np.asarray(inputs["W32"], np.float32)   # [1, 32]
    b32 = np.asarray(inputs["b32"], np.float32)

    w1t = np.zeros((P, 128), np.float32)
    for s in range(4):
        w1t[32 * s:32 * s + 10, :] = W1.T
    w2t = np.zeros((P, 64), np.float32)
    w2t[:, 0:32] = W21.T
    w2t[:, 32:64] = W22.T
    w3 = np.zeros((P, 4), np.float32)
    w3[0:32, 0:2] = W31.T
    w3[32:64, 2] = W32[0, :]
    w3[64:96, 0:2] = W31.T
    w3[96:128, 2] = W32[0, :]
    b1v = b1.reshape(P, 1)
    b2v = np.concatenate([b21, b22, b21, b22]).reshape(P, 1)
    b3 = np.array([b31[0], b31[1], b32[0]], np.float32)
    b3v = np.zeros((P, 1), np.float32)
    for g in range(4):
        b3v[32 * g:32 * g + 3, 0] = b3
    idb = np.eye(128, dtype=np.float32)
    return {
        "w1t": w1t.astype(bf), "w2t": w2t.astype(bf), "w3": w3.astype(bf),
        "idb": idb.astype(bf),
        "b1v": b1v, "b2v": b2v, "b3v": b3v,
    }


def kernel(**inputs):
    import time
    from concourse.bass_utils import run_bass_kernel_spmd
    obs = np.ascontiguousarray(np.asarray(inputs["obs"], np.float32))
    nc = _build()
    consts = _const_inputs(inputs)
    in_maps = []
    for c in range(NCORES):
        m = {"obs": obs[c * BC:(c + 1) * BC]}
        m.update(consts)
        in_maps.append(m)
    last_err = None
    for attempt in range(3):
        try:
            res = run_bass_kernel_spmd(nc, in_maps, core_ids=list(range(NCORES)))
            break
        except Exception as e:  # transient device/tunnel flakiness: retry
            last_err = e
            time.sleep(3.0)
    else:
        raise last_err
    out = np.concatenate([res.results[c]["out"] for c in range(NCORES)], axis=0)
    return out
